# revision 37
# baseline (speedup 1.0000x reference)
"""Multi-head attention (B=8, N=1024, C=768, H=12) on 8 TRN2 NeuronCores.

Sharding: data-parallel — one batch element per core, weights replicated.
No collectives.  HW exec ~211-214us vs the 589us baseline (~2.8x).

Design notes (what made it fast):
  * all matmul operands bf16 (f32 PSUM accumulate) — halves DMA + weight
    loads; rel err ~6e-3 vs the 2e-2 gate.
  * concurrent 64-row PE tiles for the S matmuls: the two heads of a
    pair live at kt/qt partitions 0:64 and 64:128, i.e. PE row-tiles T0
    and T8.  Adjacent matmuls on disjoint row tiles execute CONCURRENTLY
    (second of each pair ~4-20ns on HW, weights coexist in disjoint
    array rows).  To make the Tile scheduler keep the cross-tile pair
    adjacent, st tiles are HEAD-paired ([head A ch | head B ch], 2 PSUM
    banks): the exp then depends on both heads' S matmuls.
  * steady-state attention is Scalar-bound: one exp per (jt, ch) over
    [128,1024] PSUM (~1.0us each, 96 total); S-pair + 2 PV matmuls
    (~0.93us) hide underneath.
  * softmax normalization off the PE critical path: ones-column of the
    V stationary emits the denominator row during PV; drain per 512-col
    chunk = DVE copy of the denominator row (custom-DVE ops misread PSUM
    partition 64, so stage via plain copy) -> reciprocal_approx_fast ->
    partition_broadcast on the idle GpSimd -> DVE multiply.  ~3us after
    the last PV a bank is reusable; the baseline stalled ~35us per pair
    here (6.5us 1-partition reciprocal + 7 serial broadcast DMAs).
  * V stationary padded to M=128 columns (V | ones | zeros) so FWL
    stays enabled and the PV weight load hides under the previous MM.
  * QK projection for pair t+1 runs between pairs as PE filler while
    Scalar drains its exp backlog; its PSUM->SBUF casts run on the
    Scalar engine (idle at boundaries) — on the DVE they queue behind
    the softmax drain chain and gate the next pair's S matmuls.  PSUM
    is exactly 8 banks: 2x2-bank st/acc slots + 4 O-accumulator banks.
  * proj bias folded into the PSUM accumulation as a K=1 ones-row
    matmul; PSUM->SBUF move on the (idle by then) Scalar engine.
  * per-tile DMAs in consumption order so compute starts ~2.5us after
    the runtime's ~8us boot.

Per-core dataflow:
  phase 1: Q.T/K.T tile 0, then V[j, h, 128] via x@Wv matmuls.
  phase 2: per head pair t: S.T[j, i] = K.T_tile.T @ Q.T (K=64 row-tiled
    matmuls), exp via Scalar with fused 1/sqrt(hd) scale (logits ~N(0,1):
    no max subtraction needed), O.T[128, i] += V_aug.T @ P.T accumulated
    over j tiles; row 64 is the softmax denominator.
  phase 3: y = O.T.T @ Wp.T + bias-matmul, DMA'd out per i-tile.
"""

import numpy as np

import concourse.bacc as bacc
import concourse.mybir as mybir
import concourse.tile as tile
from concourse.bass_utils import run_bass_kernel_spmd

F32 = mybir.dt.float32
BF16 = mybir.dt.bfloat16
F32R = mybir.dt.float32r
FP8 = mybir.dt.float8e4

B, N, C = 8, 1024, 768
H, HD = 12, 64
SCALE = HD ** -0.5
NT_I = N // 128   # 8 i/j tiles
NT_C = C // 128   # 6 c tiles
NPAIR = H // 2    # 6 head pairs

MM_MODE = "bf16"
# PV matmul in fp8e4m3 DoubleRow mode: V and P quantized to fp8, two j
# subtiles contracted per instruction at 2 rows/cycle (4x the bf16 PV rate).
PV_FP8 = False


def _mm_dt(mode):
    return {"f32": F32, "f32r": F32R, "bf16": BF16}[mode]


def build(mode=MM_MODE, debug=False, pv_fp8=PV_FP8):
    dt = _mm_dt(mode)
    vdt = FP8 if pv_fp8 else dt
    nc = bacc.Bacc(None, target_bir_lowering=False)

    xt = nc.dram_tensor("xt", [C, N], dt, kind="ExternalInput")
    wq = nc.dram_tensor("wq", [C, C], dt, kind="ExternalInput")
    wk = nc.dram_tensor("wk", [C, C], dt, kind="ExternalInput")
    wv = nc.dram_tensor("wv", [C, C], dt, kind="ExternalInput")
    wp = nc.dram_tensor("wp", [C, C], dt, kind="ExternalInput")
    bias = nc.dram_tensor("bias", [128, C], F32, kind="ExternalInput")
    biasb = nc.dram_tensor("biasb", [1, C], dt, kind="ExternalInput")
    y = nc.dram_tensor("y", [N, C], F32, kind="ExternalOutput")
    if debug:
        d_r = nc.dram_tensor("d_r", [NPAIR, 2, N], F32, kind="ExternalOutput")
        d_bc = nc.dram_tensor("d_bc", [NPAIR, 2, 128, N], F32, kind="ExternalOutput")
        d_ou = nc.dram_tensor("d_ou", [2, 2, HD + 1, 512], F32, kind="ExternalOutput")
        d_pt = nc.dram_tensor("d_pt", [2, 128, N], F32, kind="ExternalOutput")
        d_qt = nc.dram_tensor("d_qt", [128, NT_C, N], F32, kind="ExternalOutput")
        d_kt = nc.dram_tensor("d_kt", [128, NT_C, N], F32, kind="ExternalOutput")
        d_v = nc.dram_tensor("d_v", [128, NT_I, H, HD + 1], F32, kind="ExternalOutput")
        d_ot = nc.dram_tensor("d_ot", [128, NT_C, N], F32, kind="ExternalOutput")

    from contextlib import ExitStack
    with tile.TileContext(nc) as tc, ExitStack() as stack:
        pp = stack.enter_context(tc.tile_pool(name="persist", bufs=1))
        ps = stack.enter_context(tc.tile_pool(name="ps", bufs=2, space="PSUM"))
        ov = stack.enter_context(tc.tile_pool(name="ov", bufs=4, space="PSUM"))
        ptp = stack.enter_context(tc.tile_pool(name="ptp", bufs=4))
        dp = stack.enter_context(tc.tile_pool(name="dp", bufs=2))
        yp = stack.enter_context(tc.tile_pool(name="yp", bufs=2))

        xt_sb = pp.tile([128, NT_C, N], dt)
        wq_sb = pp.tile([128, NT_C, C], dt)
        wk_sb = pp.tile([128, NT_C, C], dt)
        wv_sb = pp.tile([128, NT_C, C], dt)
        wp_sb = pp.tile([128, NT_C, C], dt)
        bias_sb = pp.tile([128, C], F32)
        biasb_sb = pp.tile([1, C], dt)
        ones_sb = pp.tile([1, 128], dt)
        qt_sb = pp.tile([128, NT_C, N], dt)      # Q.T rows c -> [c%128, c//128, i]
        kt_sb = pp.tile([128, NT_C, N], dt)
        # V stationary padded to 128 columns: cols 0:64 = V, col 64 = ones
        # (softmax denominator), cols 65:128 = zero pad.  M=128 keeps FWL
        # (fast weight load) enabled so the PE array never micro-idles on
        # PV weight loads — otherwise the HAM clock gate holds the whole
        # attention phase at 1.2 GHz.
        v_sb = pp.tile([128, NT_I, H, 128], vdt)
        ot_sb = pp.tile([128, NT_C, N], dt)      # normalized O.T, same tiling as qt

        # DMAs in consumption order: QK(0) streams xt k-tiles against wq/wk;
        # wv before the V matmuls; wp/bias only needed at proj time.
        for k in range(NT_C):
            nc.sync.dma_start(xt_sb[:, k, :], xt[k * 128:(k + 1) * 128, :])
            nc.sync.dma_start(wq_sb[:, k, :], wq[k * 128:(k + 1) * 128, :])
            nc.sync.dma_start(wk_sb[:, k, :], wk[k * 128:(k + 1) * 128, :])
        for k in range(NT_C):
            nc.sync.dma_start(wv_sb[:, k, :], wv[k * 128:(k + 1) * 128, :])
        nc.vector.memset(v_sb[:, :, :, HD:HD + 1], 1.0)
        nc.gpsimd.memset(v_sb[:, :, :, HD + 1:], 0.0)
        ebias_sb = pp.tile([128, 1], F32)
        nc.vector.memset(ebias_sb[:], -3.0)
        for k in range(NT_C):
            nc.sync.dma_start(wp_sb[:, k, :], wp[k * 128:(k + 1) * 128, :])
        nc.sync.dma_start(bias_sb[:], bias[:])
        nc.sync.dma_start(biasb_sb[:], biasb[:])
        nc.vector.memset(ones_sb[:], 1.0)


        def emit_qk(t):
            # Q.T / K.T for c-tile t: one [128,1024] 2-bank acc per source,
            # ch halves as independent accumulation groups; k-outer ch-inner
            # so each weight tile is reused by 2 consecutive matmuls.
            for si, (w_sb, out_sb) in enumerate(((wq_sb, qt_sb), (wk_sb, kt_sb))):
                acc = ps.tile([128, N], F32, tag="big", name=f"qk{t}_{si}")
                for k in range(NT_C):
                    for ch in range(2):
                        nc.tensor.matmul(
                            acc[:, ch * 512:(ch + 1) * 512],
                            w_sb[:, k, t * 128:(t + 1) * 128],
                            xt_sb[:, k, ch * 512:(ch + 1) * 512],
                            start=(k == 0), stop=(k == NT_C - 1),
                        )
                # Scalar, not DVE: at pair boundaries the DVE queue holds
                # the softmax drain chain; Scalar is idle there and the cast
                # gates the next pair's S matmuls.
                nc.scalar.copy(out_sb[:, t, :], acc[:])

        # ---- phase 1: QK(0), then V ----
        emit_qk(0)
        for jt in range(NT_I):
            vacc = ps.tile([128, N], F32, tag="big", name=f"v{jt}")
            for k in range(NT_C):
                for ch in range(2):
                    nc.tensor.matmul(
                        vacc[:, ch * 512:ch * 512 + 384],
                        xt_sb[:, k, jt * 128:(jt + 1) * 128],
                        wv_sb[:, k, ch * 384:(ch + 1) * 384],
                        start=(k == 0), stop=(k == NT_C - 1),
                    )
            for ch in range(2):
                nc.vector.tensor_copy(
                    v_sb[:, jt, 6 * ch:6 * ch + 6, 0:HD],
                    vacc[:, ch * 512:ch * 512 + 384].rearrange(
                        "p (h e) -> p h e", e=HD),
                )

        # ---- phase 2: attention per head pair ----
        for t in range(NPAIR):
            hA, hB = 2 * t, 2 * t + 1
            heads = ((0, hA), (64, hB))
            o = {h: [ov.tile([128, 512], F32, tag="ov", name=f"o{h}_{c}")
                     for c in range(2)]
                 for _, h in heads}
            pts = {}

            def emit_pv(jt):
                # pt(jt, ch) holds [head A ch | head B ch]; each tile feeds
                # one PV matmul per head.  Head-major order so each head's V
                # stationary is loaded once and reused by both ch matmuls.
                for hi, (_, h) in enumerate(heads):
                    for ch in range(2):
                        nc.tensor.matmul(
                            o[h][ch][:],
                            v_sb[:, jt, h, :],
                            pts[(jt, ch)][:, hi * 512:(hi + 1) * 512],
                            start=(jt == 0), stop=(jt == NT_I - 1),
                        )
                del pts[(jt, 0)], pts[(jt, 1)]

            for jt in range(NT_I):
                # Head-paired st tiles: stP(ch) = [head A ch | head B ch].
                # The exp then depends on BOTH heads' S matmuls, so the
                # scheduler keeps the A,B pair adjacent — and adjacent
                # instructions on disjoint 64-row PE tiles (T0: partitions
                # 0-63, T8: 64-127) execute CONCURRENTLY (second of each
                # pair ~4ns on HW).  The two halves land in different PSUM
                # banks of the same tile, so the concurrent writes are legal.
                for ch in range(2):
                    stp = ps.tile([128, N], F32, tag="big",
                                  name=f"st{t}_{jt}_{ch}")
                    for hi, (base, h) in enumerate(heads):
                        nc.tensor.matmul(
                            stp[:, hi * 512:(hi + 1) * 512],
                            kt_sb[base:base + 64, t, jt * 128:(jt + 1) * 128],
                            qt_sb[base:base + 64, t, ch * 512:(ch + 1) * 512],
                        )
                    pt = ptp.tile([128, N], vdt, tag="pt",
                                  name=f"pt{t}_{jt}_{ch}")
                    pts[(jt, ch)] = pt
                    nc.scalar.activation(
                        pt[:], stp[:], mybir.ActivationFunctionType.Exp,
                        scale=SCALE,
                    )
                if jt > 0:
                    emit_pv(jt - 1)
            emit_pv(NT_I - 1)

            # PE filler while Scalar drains its exp backlog for this pair.
            if t + 1 < NPAIR:
                emit_qk(t + 1)
            if debug and t == 0:
                for hi, (base, h) in enumerate(heads):
                    for c in range(2):
                        dou = dp.tile([HD + 1, 512], F32, tag="dou",
                                      name=f"dou{h}_{c}")
                        nc.vector.tensor_copy(dou[:], o[h][c][0:HD + 1, :])
                        nc.sync.dma_start(d_ou[hi, c], dou[:])

            # softmax normalization drain (off the PE critical path):
            # reciprocal of the ones-row denominator straight from PSUM,
            # broadcast across partitions on GpSimd, multiply on DVE.
            for hi, (base, h) in enumerate(heads):
                # custom-DVE ops misread PSUM partition 64 on HW: stage the
                # denominator row to SBUF partition 0 first (plain DVE copy),
                # then the fast reciprocal runs SBUF->SBUF.  Chained per
                # 512-chunk so each PSUM bank drains ~3us after its last PV.
                rl = dp.tile([1, N], F32, tag="rl", name=f"rl{t}_{h}")
                r = dp.tile([1, N], F32, tag="r", name=f"r{t}_{h}")
                bc = dp.tile([128, N], F32, tag="bc", name=f"bc{t}_{h}")
                for ch in range(2):
                    sl = slice(ch * 512, (ch + 1) * 512)
                    nc.vector.tensor_copy(rl[0:1, sl], o[h][ch][64:65, :])
                    nc.vector.reciprocal_approx_fast(r[0:1, sl], rl[0:1, sl])
                    nc.gpsimd.partition_broadcast(
                        bc[:, sl], r[0:1, sl], channels=128)
                    nc.vector.tensor_mul(
                        ot_sb[base:base + 64, t, sl],
                        o[h][ch][0:64, :],
                        bc[base:base + 64, sl],
                    )
                if debug:
                    nc.sync.dma_start(d_r[t, hi], r[0:1, :])
                    nc.sync.dma_start(d_bc[t, hi], bc[:])

        if debug:
            for src_sb, dst in ((qt_sb, d_qt), (kt_sb, d_kt), (ot_sb, d_ot)):
                dump = dp.tile([128, NT_C * N], F32, tag="dump",
                               name=f"dump_{dst.name}")
                nc.vector.tensor_copy(
                    dump[:], src_sb[:].rearrange("p a b -> p (a b)"))
                nc.sync.dma_start(dst.rearrange("p a b -> p (a b)"), dump[:])
            vdump = dp.tile([128, NT_I * H * (HD + 1)], F32, tag="dump",
                            name="vdump")
            nc.vector.tensor_copy(
                vdump[:], v_sb[:].rearrange("p a b c -> p (a b c)"))
            nc.sync.dma_start(d_v.rearrange("p a b c -> p (a b c)"), vdump[:])

        # ---- phase 3: projection + bias ----
        # bias folded into the PSUM accumulation as a K=1 ones-row matmul;
        # the PSUM->SBUF move runs on the (now idle) Scalar engine so the
        # DVE is free for the last pair's softmax drain.
        for it in range(NT_I):
            acc = ps.tile([128, N], F32, tag="big", name=f"pr{it}")
            for k in range(NT_C):
                for ch in range(2):
                    nc.tensor.matmul(
                        acc[:, ch * 512:ch * 512 + 384],
                        ot_sb[:, k, it * 128:(it + 1) * 128],
                        wp_sb[:, k, ch * 384:(ch + 1) * 384],
                        start=(k == 0), stop=False,
                    )
            for ch in range(2):
                nc.tensor.matmul(
                    acc[:, ch * 512:ch * 512 + 384],
                    ones_sb[0:1, :],
                    biasb_sb[0:1, ch * 384:(ch + 1) * 384],
                    start=False, stop=True,
                )
            y_sb = yp.tile([128, C], F32, tag="y", name=f"y{it}")
            for ch in range(2):
                nc.scalar.copy(
                    y_sb[:, ch * 384:(ch + 1) * 384],
                    acc[:, ch * 512:ch * 512 + 384],
                )
            nc.sync.dma_start(y[it * 128:(it + 1) * 128, :], y_sb[:])

    nc.compile()
    nc.finalize()
    return nc


_NC_CACHE = {}


def _get_nc(mode):
    if mode not in _NC_CACHE:
        _NC_CACHE[mode] = build(mode)
    return _NC_CACHE[mode]


def _prep_host(x, w_qkv, w_proj, b_proj, mode):
    if mode == "bf16":
        import ml_dtypes

        cast = lambda a: np.ascontiguousarray(a).astype(ml_dtypes.bfloat16)
    else:
        cast = lambda a: np.ascontiguousarray(a).astype(np.float32)
    xt = np.ascontiguousarray(x.transpose(0, 2, 1))          # [B, C, N]
    wq_t = w_qkv[0:C].T                                      # [C, C] c_in-major
    wk_t = w_qkv[C:2 * C].T
    wv_t = w_qkv[2 * C:3 * C].T
    wp_t = w_proj.T
    bias_rep = np.ascontiguousarray(
        np.broadcast_to(b_proj.astype(np.float32), (128, C))
    )
    biasb = cast(np.asarray(b_proj).reshape(1, C))
    return (cast(xt), cast(wq_t), cast(wk_t), cast(wv_t), cast(wp_t), bias_rep,
            biasb)


def run(x, w_qkv, w_proj, b_proj, mode=MM_MODE, trace=False):
    nc = _get_nc(mode)
    xt, wq_t, wk_t, wv_t, wp_t, bias_rep, biasb = _prep_host(
        x, w_qkv, w_proj, b_proj, mode)
    in_maps = [
        {"xt": np.ascontiguousarray(xt[b]), "wq": wq_t, "wk": wk_t,
         "wv": wv_t, "wp": wp_t, "bias": bias_rep, "biasb": biasb}
        for b in range(B)
    ]
    res = run_bass_kernel_spmd(
        nc, in_maps, core_ids=list(range(B)), trace=trace
    )
    out = np.stack([res.results[b]["y"] for b in range(B)]).astype(np.float32)
    return out, res


def kernel(x, w_qkv, w_proj, b_proj):
    out, _ = run(x, w_qkv, w_proj, b_proj)
    return out


# revision 38
# speedup vs baseline: 1.0202x; 1.0202x over previous
"""Multi-head attention (B=8, N=1024, C=768, H=12) on 8 TRN2 NeuronCores.

Sharding: data-parallel — one batch element per core, weights replicated.
No collectives.  HW exec ~211-214us vs the 589us baseline (~2.8x).

Design notes (what made it fast):
  * all matmul operands bf16 (f32 PSUM accumulate) — halves DMA + weight
    loads; rel err ~6e-3 vs the 2e-2 gate.
  * concurrent 64-row PE tiles for the S matmuls: the two heads of a
    pair live at kt/qt partitions 0:64 and 64:128, i.e. PE row-tiles T0
    and T8.  Adjacent matmuls on disjoint row tiles execute CONCURRENTLY
    (second of each pair ~4-20ns on HW, weights coexist in disjoint
    array rows).  To make the Tile scheduler keep the cross-tile pair
    adjacent, st tiles are HEAD-paired ([head A ch | head B ch], 2 PSUM
    banks): the exp then depends on both heads' S matmuls.
  * steady-state attention is Scalar-bound: one exp per (jt, ch) over
    [128,1024] PSUM (~1.0us each, 96 total); S-pair + 2 PV matmuls
    (~0.93us) hide underneath.
  * softmax normalization off the PE critical path: ones-column of the
    V stationary emits the denominator row during PV; drain per 512-col
    chunk = DVE copy of the denominator row (custom-DVE ops misread PSUM
    partition 64, so stage via plain copy) -> reciprocal_approx_fast ->
    partition_broadcast on the idle GpSimd -> DVE multiply.  ~3us after
    the last PV a bank is reusable; the baseline stalled ~35us per pair
    here (6.5us 1-partition reciprocal + 7 serial broadcast DMAs).
  * V stationary padded to M=128 columns (V | ones | zeros) so FWL
    stays enabled and the PV weight load hides under the previous MM.
  * QK projection for pair t+1 runs between pairs as PE filler while
    Scalar drains its exp backlog; its PSUM->SBUF casts run on the
    Scalar engine (idle at boundaries) — on the DVE they queue behind
    the softmax drain chain and gate the next pair's S matmuls.  PSUM
    is exactly 8 banks: 2x2-bank st/acc slots + 4 O-accumulator banks.
  * proj bias folded into the PSUM accumulation as a K=1 ones-row
    matmul; PSUM->SBUF move on the (idle by then) Scalar engine.
  * per-tile DMAs in consumption order so compute starts ~2.5us after
    the runtime's ~8us boot.

Per-core dataflow:
  phase 1: Q.T/K.T tile 0, then V[j, h, 128] via x@Wv matmuls.
  phase 2: per head pair t: S.T[j, i] = K.T_tile.T @ Q.T (K=64 row-tiled
    matmuls), exp via Scalar with fused 1/sqrt(hd) scale (logits ~N(0,1):
    no max subtraction needed), O.T[128, i] += V_aug.T @ P.T accumulated
    over j tiles; row 64 is the softmax denominator.
  phase 3: y = O.T.T @ Wp.T + bias-matmul, DMA'd out per i-tile.
"""

import numpy as np

import concourse.bacc as bacc
import concourse.mybir as mybir
import concourse.tile as tile
from concourse.bass_utils import run_bass_kernel_spmd

F32 = mybir.dt.float32
BF16 = mybir.dt.bfloat16
F32R = mybir.dt.float32r
FP8 = mybir.dt.float8e4

B, N, C = 8, 1024, 768
H, HD = 12, 64
SCALE = HD ** -0.5
NT_I = N // 128   # 8 i/j tiles
NT_C = C // 128   # 6 c tiles
NPAIR = H // 2    # 6 head pairs

MM_MODE = "bf16"
# PV matmul in fp8e4m3 DoubleRow mode: V and P quantized to fp8, two j
# subtiles contracted per instruction at 2 rows/cycle (4x the bf16 PV rate).
PV_FP8 = False


def _mm_dt(mode):
    return {"f32": F32, "f32r": F32R, "bf16": BF16}[mode]


def build(mode=MM_MODE, debug=False, pv_fp8=PV_FP8):
    dt = _mm_dt(mode)
    vdt = FP8 if pv_fp8 else dt
    nc = bacc.Bacc(None, target_bir_lowering=False)

    xt = nc.dram_tensor("xt", [C, N], dt, kind="ExternalInput")
    wq = nc.dram_tensor("wq", [C, C], dt, kind="ExternalInput")
    wk = nc.dram_tensor("wk", [C, C], dt, kind="ExternalInput")
    wv = nc.dram_tensor("wv", [C, C], dt, kind="ExternalInput")
    wp = nc.dram_tensor("wp", [C, C], dt, kind="ExternalInput")
    bias = nc.dram_tensor("bias", [128, C], F32, kind="ExternalInput")
    biasb = nc.dram_tensor("biasb", [1, C], dt, kind="ExternalInput")
    y = nc.dram_tensor("y", [N, C], F32, kind="ExternalOutput")
    if debug:
        d_r = nc.dram_tensor("d_r", [NPAIR, 2, N], F32, kind="ExternalOutput")
        d_bc = nc.dram_tensor("d_bc", [NPAIR, 2, 128, N], F32, kind="ExternalOutput")
        d_ou = nc.dram_tensor("d_ou", [2, 2, HD + 1, 512], F32, kind="ExternalOutput")
        d_pt = nc.dram_tensor("d_pt", [2, 128, N], F32, kind="ExternalOutput")
        d_qt = nc.dram_tensor("d_qt", [128, NT_C, N], F32, kind="ExternalOutput")
        d_kt = nc.dram_tensor("d_kt", [128, NT_C, N], F32, kind="ExternalOutput")
        d_v = nc.dram_tensor("d_v", [128, NT_I, H, HD + 1], F32, kind="ExternalOutput")
        d_ot = nc.dram_tensor("d_ot", [128, NT_C, N], F32, kind="ExternalOutput")

    from contextlib import ExitStack
    with tile.TileContext(nc) as tc, ExitStack() as stack:
        pp = stack.enter_context(tc.tile_pool(name="persist", bufs=1))
        ps = stack.enter_context(tc.tile_pool(name="ps", bufs=2, space="PSUM"))
        ov = stack.enter_context(tc.tile_pool(name="ov", bufs=4, space="PSUM"))
        ptp = stack.enter_context(tc.tile_pool(name="ptp", bufs=4))
        dp = stack.enter_context(tc.tile_pool(name="dp", bufs=2))
        yp = stack.enter_context(tc.tile_pool(name="yp", bufs=2))

        xt_sb = pp.tile([128, NT_C, N], dt)
        wq_sb = pp.tile([128, NT_C, C], dt)
        wk_sb = pp.tile([128, NT_C, C], dt)
        wv_sb = pp.tile([128, NT_C, C], dt)
        wp_sb = pp.tile([128, NT_C, C], dt)
        bias_sb = pp.tile([128, C], F32)
        biasb_sb = pp.tile([1, C], dt)
        ones_sb = pp.tile([1, 128], dt)
        qt_sb = pp.tile([128, NT_C, N], dt)      # Q.T rows c -> [c%128, c//128, i]
        kt_sb = pp.tile([128, NT_C, N], dt)
        # V stationary padded to 128 columns: cols 0:64 = V, col 64 = ones
        # (softmax denominator), cols 65:128 = zero pad.  M=128 keeps FWL
        # (fast weight load) enabled so the PE array never micro-idles on
        # PV weight loads — otherwise the HAM clock gate holds the whole
        # attention phase at 1.2 GHz.
        v_sb = pp.tile([128, NT_I, H, 128], vdt)
        ot_sb = pp.tile([128, NT_C, N], dt)      # normalized O.T, same tiling as qt

        # DMAs in consumption order: QK(0) streams xt k-tiles against wq/wk;
        # wv before the V matmuls; wp/bias only needed at proj time.
        for k in range(NT_C):
            nc.sync.dma_start(xt_sb[:, k, :], xt[k * 128:(k + 1) * 128, :])
            nc.sync.dma_start(wv_sb[:, k, :], wv[k * 128:(k + 1) * 128, :])
        for k in range(NT_C):
            nc.sync.dma_start(wq_sb[:, k, :], wq[k * 128:(k + 1) * 128, :])
            nc.sync.dma_start(wk_sb[:, k, :], wk[k * 128:(k + 1) * 128, :])
        nc.vector.memset(v_sb[:, :, :, HD:HD + 1], 1.0)
        nc.gpsimd.memset(v_sb[:, :, :, HD + 1:], 0.0)
        ebias_sb = pp.tile([128, 1], F32)
        nc.vector.memset(ebias_sb[:], -3.0)
        for k in range(NT_C):
            nc.sync.dma_start(wp_sb[:, k, :], wp[k * 128:(k + 1) * 128, :])
        nc.sync.dma_start(bias_sb[:], bias[:])
        nc.sync.dma_start(biasb_sb[:], biasb[:])
        nc.vector.memset(ones_sb[:], 1.0)


        def emit_qk(t):
            # Q.T / K.T for c-tile t: one [128,1024] 2-bank acc per source,
            # ch halves as independent accumulation groups; k-outer ch-inner
            # so each weight tile is reused by 2 consecutive matmuls.
            for si, (w_sb, out_sb) in enumerate(((wq_sb, qt_sb), (wk_sb, kt_sb))):
                acc = ps.tile([128, N], F32, tag="big", name=f"qk{t}_{si}")
                for k in range(NT_C):
                    for ch in range(2):
                        nc.tensor.matmul(
                            acc[:, ch * 512:(ch + 1) * 512],
                            w_sb[:, k, t * 128:(t + 1) * 128],
                            xt_sb[:, k, ch * 512:(ch + 1) * 512],
                            start=(k == 0), stop=(k == NT_C - 1),
                        )
                # Scalar, not DVE: at pair boundaries the DVE queue holds
                # the softmax drain chain; Scalar is idle there and the cast
                # gates the next pair's S matmuls.
                nc.scalar.copy(out_sb[:, t, :], acc[:])

        # ---- phase 1: V first (needs only xt+wv DMA, ~2.6MB), then QK(0)
        # (needs wq/wk as well) — first exp starts ~13us earlier.
        for jt in range(NT_I):
            vacc = ps.tile([128, N], F32, tag="big", name=f"v{jt}")
            for k in range(NT_C):
                for ch in range(2):
                    nc.tensor.matmul(
                        vacc[:, ch * 512:ch * 512 + 384],
                        xt_sb[:, k, jt * 128:(jt + 1) * 128],
                        wv_sb[:, k, ch * 384:(ch + 1) * 384],
                        start=(k == 0), stop=(k == NT_C - 1),
                    )
            for ch in range(2):
                nc.vector.tensor_copy(
                    v_sb[:, jt, 6 * ch:6 * ch + 6, 0:HD],
                    vacc[:, ch * 512:ch * 512 + 384].rearrange(
                        "p (h e) -> p h e", e=HD),
                )
        emit_qk(0)

        # ---- phase 2: attention per head pair ----
        for t in range(NPAIR):
            hA, hB = 2 * t, 2 * t + 1
            heads = ((0, hA), (64, hB))
            o = {h: [ov.tile([128, 512], F32, tag="ov", name=f"o{h}_{c}")
                     for c in range(2)]
                 for _, h in heads}
            pts = {}

            def emit_pv(jt):
                # pt(jt, ch) holds [head A ch | head B ch]; each tile feeds
                # one PV matmul per head.  Head-major order so each head's V
                # stationary is loaded once and reused by both ch matmuls.
                for hi, (_, h) in enumerate(heads):
                    for ch in range(2):
                        nc.tensor.matmul(
                            o[h][ch][:],
                            v_sb[:, jt, h, :],
                            pts[(jt, ch)][:, hi * 512:(hi + 1) * 512],
                            start=(jt == 0), stop=(jt == NT_I - 1),
                        )
                del pts[(jt, 0)], pts[(jt, 1)]

            for jt in range(NT_I):
                # Head-paired st tiles: stP(ch) = [head A ch | head B ch].
                # The exp then depends on BOTH heads' S matmuls, so the
                # scheduler keeps the A,B pair adjacent — and adjacent
                # instructions on disjoint 64-row PE tiles (T0: partitions
                # 0-63, T8: 64-127) execute CONCURRENTLY (second of each
                # pair ~4ns on HW).  The two halves land in different PSUM
                # banks of the same tile, so the concurrent writes are legal.
                for ch in range(2):
                    stp = ps.tile([128, N], F32, tag="big",
                                  name=f"st{t}_{jt}_{ch}")
                    for hi, (base, h) in enumerate(heads):
                        nc.tensor.matmul(
                            stp[:, hi * 512:(hi + 1) * 512],
                            kt_sb[base:base + 64, t, jt * 128:(jt + 1) * 128],
                            qt_sb[base:base + 64, t, ch * 512:(ch + 1) * 512],
                        )
                    pt = ptp.tile([128, N], vdt, tag="pt",
                                  name=f"pt{t}_{jt}_{ch}")
                    pts[(jt, ch)] = pt
                    nc.scalar.activation(
                        pt[:], stp[:], mybir.ActivationFunctionType.Exp,
                        scale=SCALE,
                    )
                if jt > 0:
                    emit_pv(jt - 1)
            emit_pv(NT_I - 1)

            # PE filler while Scalar drains its exp backlog for this pair.
            if t + 1 < NPAIR:
                emit_qk(t + 1)
            if debug and t == 0:
                for hi, (base, h) in enumerate(heads):
                    for c in range(2):
                        dou = dp.tile([HD + 1, 512], F32, tag="dou",
                                      name=f"dou{h}_{c}")
                        nc.vector.tensor_copy(dou[:], o[h][c][0:HD + 1, :])
                        nc.sync.dma_start(d_ou[hi, c], dou[:])

            # softmax normalization drain (off the PE critical path):
            # reciprocal of the ones-row denominator straight from PSUM,
            # broadcast across partitions on GpSimd, multiply on DVE.
            for hi, (base, h) in enumerate(heads):
                # custom-DVE ops misread PSUM partition 64 on HW: stage the
                # denominator row to SBUF partition 0 first (plain DVE copy),
                # then the fast reciprocal runs SBUF->SBUF.  Chained per
                # 512-chunk so each PSUM bank drains ~3us after its last PV.
                rl = dp.tile([1, N], F32, tag="rl", name=f"rl{t}_{h}")
                r = dp.tile([1, N], F32, tag="r", name=f"r{t}_{h}")
                bc = dp.tile([128, N], F32, tag="bc", name=f"bc{t}_{h}")
                for ch in range(2):
                    sl = slice(ch * 512, (ch + 1) * 512)
                    if t == NPAIR - 1:
                        nc.scalar.copy(rl[0:1, sl], o[h][ch][64:65, :])
                    else:
                        nc.vector.tensor_copy(rl[0:1, sl], o[h][ch][64:65, :])
                    nc.vector.reciprocal_approx_fast(r[0:1, sl], rl[0:1, sl])
                    nc.gpsimd.partition_broadcast(
                        bc[:, sl], r[0:1, sl], channels=128)
                    nc.vector.tensor_mul(
                        ot_sb[base:base + 64, t, sl],
                        o[h][ch][0:64, :],
                        bc[base:base + 64, sl],
                    )
                if debug:
                    nc.sync.dma_start(d_r[t, hi], r[0:1, :])
                    nc.sync.dma_start(d_bc[t, hi], bc[:])

        if debug:
            for src_sb, dst in ((qt_sb, d_qt), (kt_sb, d_kt), (ot_sb, d_ot)):
                dump = dp.tile([128, NT_C * N], F32, tag="dump",
                               name=f"dump_{dst.name}")
                nc.vector.tensor_copy(
                    dump[:], src_sb[:].rearrange("p a b -> p (a b)"))
                nc.sync.dma_start(dst.rearrange("p a b -> p (a b)"), dump[:])
            vdump = dp.tile([128, NT_I * H * (HD + 1)], F32, tag="dump",
                            name="vdump")
            nc.vector.tensor_copy(
                vdump[:], v_sb[:].rearrange("p a b c -> p (a b c)"))
            nc.sync.dma_start(d_v.rearrange("p a b c -> p (a b c)"), vdump[:])

        # ---- phase 3: projection + bias ----
        # bias folded into the PSUM accumulation as a K=1 ones-row matmul;
        # the PSUM->SBUF move runs on the (now idle) Scalar engine so the
        # DVE is free for the last pair's softmax drain.
        for it in range(NT_I):
            acc = ps.tile([128, N], F32, tag="big", name=f"pr{it}")
            for k in range(NT_C):
                for ch in range(2):
                    nc.tensor.matmul(
                        acc[:, ch * 512:ch * 512 + 384],
                        ot_sb[:, k, it * 128:(it + 1) * 128],
                        wp_sb[:, k, ch * 384:(ch + 1) * 384],
                        start=(k == 0), stop=False,
                    )
            for ch in range(2):
                nc.tensor.matmul(
                    acc[:, ch * 512:ch * 512 + 384],
                    ones_sb[0:1, :],
                    biasb_sb[0:1, ch * 384:(ch + 1) * 384],
                    start=False, stop=True,
                )
            y_sb = yp.tile([128, C], F32, tag="y", name=f"y{it}")
            for ch in range(2):
                nc.scalar.copy(
                    y_sb[:, ch * 384:(ch + 1) * 384],
                    acc[:, ch * 512:ch * 512 + 384],
                )
            nc.sync.dma_start(y[it * 128:(it + 1) * 128, :], y_sb[:])

    nc.compile()
    nc.finalize()
    return nc


_NC_CACHE = {}


def _get_nc(mode):
    if mode not in _NC_CACHE:
        _NC_CACHE[mode] = build(mode)
    return _NC_CACHE[mode]


def _prep_host(x, w_qkv, w_proj, b_proj, mode):
    if mode == "bf16":
        import ml_dtypes

        cast = lambda a: np.ascontiguousarray(a).astype(ml_dtypes.bfloat16)
    else:
        cast = lambda a: np.ascontiguousarray(a).astype(np.float32)
    xt = np.ascontiguousarray(x.transpose(0, 2, 1))          # [B, C, N]
    wq_t = w_qkv[0:C].T                                      # [C, C] c_in-major
    wk_t = w_qkv[C:2 * C].T
    wv_t = w_qkv[2 * C:3 * C].T
    wp_t = w_proj.T
    bias_rep = np.ascontiguousarray(
        np.broadcast_to(b_proj.astype(np.float32), (128, C))
    )
    biasb = cast(np.asarray(b_proj).reshape(1, C))
    return (cast(xt), cast(wq_t), cast(wk_t), cast(wv_t), cast(wp_t), bias_rep,
            biasb)


def run(x, w_qkv, w_proj, b_proj, mode=MM_MODE, trace=False):
    nc = _get_nc(mode)
    xt, wq_t, wk_t, wv_t, wp_t, bias_rep, biasb = _prep_host(
        x, w_qkv, w_proj, b_proj, mode)
    in_maps = [
        {"xt": np.ascontiguousarray(xt[b]), "wq": wq_t, "wk": wk_t,
         "wv": wv_t, "wp": wp_t, "bias": bias_rep, "biasb": biasb}
        for b in range(B)
    ]
    res = run_bass_kernel_spmd(
        nc, in_maps, core_ids=list(range(B)), trace=trace
    )
    out = np.stack([res.results[b]["y"] for b in range(B)]).astype(np.float32)
    return out, res


def kernel(x, w_qkv, w_proj, b_proj):
    out, _ = run(x, w_qkv, w_proj, b_proj)
    return out
